# revision 52
# baseline (speedup 1.0000x reference)
"""Trainium2 Bass kernel for nn_Attention_4363686773373.

Sigmoid attention with magnitude-preserving (weight-normalized) projections.

Sharding: data-parallel over (batch, T-half) -> 8 shards on 8 NeuronCores.
Each core computes q for its 1024 tokens and k,v for the full 2048 tokens of
its batch (k/v recomputed on both cores; no collectives). Each core's xkv
rows are pre-ordered so its query tokens come first (attention is
permutation-invariant over the key axis), keeping the program SPMD-uniform.

Per-core dataflow (heavy matmuls in bf16 with fp32 PSUM accumulation):
  W/X: row-normalize qkv_w on device and PE-transpose it (and the bf16-cast
     x) into [d, .] layouts; out_w is normalized and bounced through DRAM
     with large DMA-xbar transposes (only needed by the out-projection);
     per-token ||x|| via ACT square+accumulate.
  A: qkv projection in natural [t, e] layout (lhsT = xT tiles), fast PSUM
     eviction through ACT copies, q/k cosine-normalization along head_dim
     via free-dim reduces, then DRAM-bounce transposes to [head_dim, t]
     layout. q token-blocks are interleaved into the k/v loop.
  B: unit = (head-pair, t-half, key-block), t-half-major so early units only
     need early qn transposes. Both heads' K=64 score matmuls land side by
     side in one [128, 1024] PSUM tile (adjacent issue -> concurrent in
     disjoint PE row groups); ONE FD=1024 sigmoid on the scalar engine
     yields bf16 attn weights; attn^T @ v accumulates per pair. Score tiles
     are triple-buffered and issued two units ahead so the PE never stalls
     inside a sigmoid (keeps the PE HAM clock warm -- the biggest lever
     observed: a cold-entering phase B stays at half clock for its entire
     duration). Per-pair attn-out transposes run on the idle sync xbar ring.
  C: software-pipelined per token-block: normalize per (token, head), scale
     by token magnitude, PE-transpose to [d, t], out-projection, store.

DMA rings: sync = loads + xbar transposes, scalar = weight loads, gpsimd =
DRAM scratch writes + output stores. ACT table sets: all sqrt-set work
strictly precedes the sigmoid phase; one switch back for phase C's sqrt
(2 table switches total). PE-transpose batches land in a single-bank PSUM
tile and are evicted by one strided DVE copy. Measured ~422 us on 8
axon-tunneled trn2 cores, rel err 4.5e-3 vs the fp32 reference.
"""

import math
from contextlib import ExitStack

import numpy as np

import concourse.bass as bass
import concourse.tile as tile
from concourse import bacc, mybir
from concourse.bass_utils import run_bass_kernel_spmd
from concourse.masks import make_identity

# Problem shapes (hardcoded per harness contract)
B, T, D, H = 4, 2048, 768, 12
HD = D // H  # 64
EPS = 1e-4
SIGMOID_GAIN = 1.8402
N_CORES = 8

F32 = mybir.dt.float32
BF16 = mybir.dt.bfloat16
AF = mybir.ActivationFunctionType
ALU = mybir.AluOpType
AX = mybir.AxisListType


def _ensure_axon_hooks():
    """This image's antenv lacks axon_hooks; reconstruct it so trace=True
    (NTFF profiling) works instead of crashing on import."""
    try:
        import antenv.axon_hooks  # noqa: F401
        return
    except ImportError:
        pass
    import sys
    import types
    try:
        import antenv
    except ImportError:
        return
    mod = types.ModuleType("antenv.axon_hooks")
    _hook = [None]
    mod.set_axon_ntff_profile_hook = lambda h: _hook.__setitem__(0, h)
    mod.get_axon_ntff_profile_hook = lambda: _hook[0]
    sys.modules["antenv.axon_hooks"] = mod
    antenv.axon_hooks = mod
    try:
        from trn_agent_boot.trn_boot import _ntff_profile_via_ctypes
        mod.set_axon_ntff_profile_hook(
            _ntff_profile_via_ctypes('/opt/axon/libaxon_pjrt.so'))
    except Exception:
        pass


_ensure_axon_hooks()

if __import__("os").environ.get("ANT_LDW_OPT") == "1":
    import concourse.bass_utils as _bu
    _orig_rc = _bu.run_command

    def _rc_ldw(argv, **kw):
        argv = ["--enable-ldw-opt=true" if a == "--enable-ldw-opt=false" else a
                for a in argv]
        return _orig_rc(argv, **kw)

    _bu.run_command = _rc_ldw


def _chunks(total, maxn=1024):
    out = []
    c0 = 0
    while c0 < total:
        cn = min(maxn, total - c0)
        out.append((c0, cn))
        c0 += cn
    return out


def build_program(nc, tc, ctx, Tq, Tkv, Dm, Hn):
    """Emit the per-core program. xkv rows are pre-ordered so the first Tq
    tokens are this core's query tokens (attention is permutation-invariant
    over the key axis)."""
    keep = []  # keep tc.tile free-closures alive (GC would release the pools)

    def _tile(shape, dtype, name):
        t, free = tc.tile(shape, dtype, name=name)
        keep.append(free)
        return t, free

    tc._ant_keepalive = keep
    P = 128
    HDl = 64
    assert Dm % P == 0 and Tq % P == 0 and Tkv % P == 0
    DT = Dm // P          # d-tiles
    E3 = 3 * Dm
    PAIRS = Hn // 2       # head pairs; pair = 128 contiguous features
    assert PAIRS * P == Dm and Hn * HDl == Dm
    TBq = Tq // P
    TBkv = Tkv // P
    WE = E3 // P          # qkv_w row tiles
    # eps seen by the post-attention normalize, after folding out the
    # gain/sqrt(T) prefactor (we accumulate raw attn@v).
    eps_av = EPS * math.sqrt(Tkv) / SIGMOID_GAIN

    xkv = nc.dram_tensor("xkv", [Tkv, Dm], F32, kind="ExternalInput").ap()
    qkvw = nc.dram_tensor("qkvw", [E3, Dm], F32, kind="ExternalInput").ap()
    outw = nc.dram_tensor("outw", [Dm, Dm], F32, kind="ExternalInput").ap()
    y = nc.dram_tensor("y", [Tq, Dm], F32, kind="ExternalOutput").ap()

    # ---------------- DRAM scratch ----------------
    dstk = ExitStack()
    dpool = dstk.enter_context(tc.tile_pool(name="dram", bufs=1, space="DRAM"))
    own_dram = dpool.tile([Dm, Dm], BF16, name="own_dram")
    kn_dram = dpool.tile([Tkv, Dm], BF16, name="kn_dram")
    qn_dram = dpool.tile([Tq, Dm], BF16, name="qn_dram")

    # ---------------- persistent SBUF tensors ----------------
    knT, _ = _tile([P, PAIRS * Tkv], BF16, "knT")    # [hd(2 heads), s]
    qnT, _ = _tile([P, PAIRS * Tq], BF16, "qnT")     # [hd(2 heads), t]
    vbig, _ = _tile([P, TBkv * Dm], BF16, "vbig")    # natural [s, e]
    mag8, _ = _tile([P, max(TBq, 2)], F32, "mag8")   # sqrt(||x||^2*HD/D)
    ownT, _ = _tile([P, DT * Dm], BF16, "ownT")      # out_w normalized^T
    avnat, _ = _tile([P, TBq * Dm], BF16, "avnat")   # attn-out natural
    ident, _ = _tile([P, P], BF16, "ident")          # PE-transpose identity
    make_identity(nc, ident)

    # ---------------- phase W + X + A (scoped) ----------------
    wxa = ExitStack()
    wnT, free_wnT = _tile([P, DT * E3], BF16, "wnT")
    xkvT, free_xkvT = _tile([P, DT * Tkv], BF16, "xkvT")
    wstage = wxa.enter_context(tc.tile_pool(name="wstage", bufs=6))
    sqpool = wxa.enter_context(tc.tile_pool(name="sqpool", bufs=4))
    small = wxa.enter_context(tc.tile_pool(name="small", bufs=24))
    nstage = wxa.enter_context(tc.tile_pool(name="nstage", bufs=6))
    psA = wxa.enter_context(tc.tile_pool(name="psA", bufs=2, space="PSUM"))
    psW = wxa.enter_context(tc.tile_pool(name="psW", bufs=2, space="PSUM"))

    def pe_transpose_cols(src, dst_big, cols, stride, base):
        """PE-transpose src [P, DT*P] column blocks into dst_big where block
        dt lands at dst_big[:, dt*stride + base : +cols]. All DT transposes
        land in one single-bank PSUM tile, evicted by ONE strided DVE copy."""
        ptw = psW.tile([P, DT * P], BF16, name="ptw", tag="ptw")
        for dt in range(DT):
            nc.tensor.transpose(ptw[:, dt * P:(dt + 1) * P],
                                src[:, dt * P:(dt + 1) * P], ident,
                                )
        dst3 = dst_big.rearrange("p (dt s) -> p dt s", dt=DT)[:, :, base:base + cols]
        nc.vector.tensor_copy(dst3, ptw.rearrange("p (dt s) -> p dt s", dt=DT))

    def normalize_w(we):
        """qkv_w row-tile we -> bf16 rows/(||row||+eps), PE-transposed into
        wnT."""
        wst = wstage.tile([P, Dm], F32, name="wst", tag="wst")
        nc.scalar.dma_start(wst, qkvw[we * P:(we + 1) * P, :])
        wsq = sqpool.tile([P, Dm], BF16, name="wsq", tag="sq")
        ssw = small.tile([P, 1], F32, name="ssw", tag="s1")
        nc.scalar.activation(wsq, wst, AF.Square, accum_out=ssw)
        sw = small.tile([P, 1], F32, name="sw", tag="s1")
        nc.scalar.activation(sw, ssw, AF.Sqrt)
        swe = small.tile([P, 1], F32, name="swe", tag="s1")
        nc.vector.tensor_scalar_add(swe, sw, EPS)
        rw = small.tile([P, 1], F32, name="rw", tag="s1")
        nc.vector.reciprocal(rw, swe)
        wnb = nstage.tile([P, Dm], BF16, name="wnb", tag="nst")
        nc.vector.tensor_scalar_mul(wnb, wst, rw)
        pe_transpose_cols(wnb, wnT, P, E3, we * P)

    def load_x(ti):
        """x token-block ti: magnitude, bf16 cast, PE-transpose into xkvT."""
        xst = wstage.tile([P, Dm], F32, name="xst", tag="wst")
        nc.sync.dma_start(xst, xkv[ti * P:(ti + 1) * P, :])
        if ti < TBq:
            xsq = sqpool.tile([P, Dm], BF16, name="xsq", tag="sq")
            ssx = small.tile([P, 1], F32, name="ssx", tag="s1")
            nc.scalar.activation(xsq, xst, AF.Square, accum_out=ssx)
            nc.scalar.activation(mag8[:, ti:ti + 1], ssx, AF.Sqrt,
                                 scale=float(HDl) / float(Dm))
        xbf = nstage.tile([P, Dm], BF16, name="xbf", tag="nst")
        nc.vector.tensor_copy(xbf, xst)
        pe_transpose_cols(xbf, xkvT, P, Tkv, ti * P)

    # interleave x blocks with k/v weight rows (rows Dm..3Dm); phase A's kv
    # loop needs all kv weight tiles + per-ti x tiles
    for i in range(max(TBkv, 2 * DT)):
        if i < TBkv:
            load_x(i)
        if i < 2 * DT:
            normalize_w(DT + i)
    for we in range(DT):     # q weight rows last (q loop runs after kv loop)
        normalize_w(we)

    # out-projection weights: normalize -> DRAM bounce -> xbar transpose
    # (only needed by phase C; uses idle DMA capacity during A/B)
    for we in range(DT):
        wst = wstage.tile([P, Dm], F32, name="wso", tag="wst")
        nc.scalar.dma_start(wst, outw[we * P:(we + 1) * P, :])
        wsq = sqpool.tile([P, Dm], BF16, name="wsqo", tag="sq")
        ssw = small.tile([P, 1], F32, name="sswo", tag="s1")
        nc.scalar.activation(wsq, wst, AF.Square, accum_out=ssw)
        sw = small.tile([P, 1], F32, name="swo", tag="s1")
        nc.scalar.activation(sw, ssw, AF.Sqrt)
        swe = small.tile([P, 1], F32, name="sweo", tag="s1")
        nc.vector.tensor_scalar_add(swe, sw, EPS)
        rw = small.tile([P, 1], F32, name="rwo", tag="s1")
        nc.vector.reciprocal(rw, swe)
        wnb = nstage.tile([P, Dm], BF16, name="wnbo", tag="nst")
        nc.vector.tensor_scalar_mul(wnb, wst, rw)
        nc.gpsimd.dma_start(own_dram[we * P:(we + 1) * P, :], wnb)
    for dt in range(DT):
        nc.sync.dma_start_transpose(
            ownT[:, dt * Dm:(dt + 1) * Dm],
            own_dram[:, dt * P:(dt + 1) * P])

    # qkv projection + q/k normalization, natural layout
    def qk_normalize(kraw, is_k):
        """kraw: SBUF bf16 [P, Dm] raw q or k; returns normalized bf16 tile."""
        sqk = sqpool.tile([P, Dm], BF16, name="sqk", tag="sq")
        nc.vector.tensor_mul(sqk, kraw, kraw)
        ssk = small.tile([P, Hn], F32, name="ssk", tag="sh")
        nc.vector.tensor_reduce(ssk, sqk.rearrange("p (h d) -> p h d", h=Hn),
                                axis=AX.X, op=ALU.add)
        sk = small.tile([P, Hn], F32, name="sk", tag="sh")
        nc.scalar.activation(sk, ssk, AF.Sqrt)
        ske = small.tile([P, Hn], F32, name="ske", tag="sh")
        if is_k:
            # fold the 1/sqrt(HD) score scale into k: sqrt(HD)/(||k||+eps)
            nc.vector.tensor_scalar(ske, sk, EPS, 1.0 / math.sqrt(HDl),
                                    op0=ALU.add, op1=ALU.mult)
        else:
            nc.vector.tensor_scalar_add(ske, sk, EPS)
        rk = small.tile([P, Hn], F32, name="rk", tag="sh")
        nc.vector.reciprocal(rk, ske)
        knb = nstage.tile([P, Dm], BF16, name="knb", tag="nst")
        nc.vector.tensor_tensor(
            knb.rearrange("p (h d) -> p h d", h=Hn),
            kraw.rearrange("p (h d) -> p h d", h=Hn),
            rk.broadcast_to([P, Hn, HDl]),
            op=ALU.mult)
        return knb

    def emit_q(ti):
        # q for this core's token blocks (first TBq blocks of xkv)
        ps = psA.tile([P, Dm], F32, name="psq", tag="ps")
        for dt in range(DT):
            lhs = xkvT[:, dt * Tkv + ti * P: dt * Tkv + (ti + 1) * P]
            for (c0, cn) in _chunks(Dm, 512):
                nc.tensor.matmul(ps[:, c0:c0 + cn], lhsT=lhs,
                                 rhs=wnT[:, dt * E3 + c0: dt * E3 + c0 + cn],
                                 start=(dt == 0), stop=(dt == DT - 1))
        qraw = sqpool.tile([P, Dm], BF16, name="qraw", tag="kraw")
        nc.scalar.activation(qraw, ps[:, 0:Dm], AF.Copy)
        qnb = qk_normalize(qraw, False)
        nc.gpsimd.dma_start(qn_dram[ti * P:(ti + 1) * P, :], qnb)
        QH = max(TBq // 2, 1)
        if ti % QH == QH - 1:
            h0 = (ti // QH) * QH * P
            hn = QH * P
            for pr in range(PAIRS):
                nc.sync.dma_start_transpose(
                    qnT[:, pr * Tq + h0: pr * Tq + h0 + hn],
                    qn_dram[h0:h0 + hn, pr * P:(pr + 1) * P])

    KQ = max(TBkv // 4, 1)
    qdone = 0
    for ti in range(TBkv):
        # k,v for every token block
        ps = psA.tile([P, 2 * Dm], F32, name="pskv", tag="ps")
        for dt in range(DT):
            lhs = xkvT[:, dt * Tkv + ti * P: dt * Tkv + (ti + 1) * P]
            for (c0, cn) in _chunks(2 * Dm, 512):
                nc.tensor.matmul(ps[:, c0:c0 + cn], lhsT=lhs,
                                 rhs=wnT[:, dt * E3 + Dm + c0: dt * E3 + Dm + c0 + cn],
                                 start=(dt == 0), stop=(dt == DT - 1))
        # evict PSUM quickly (frees the accumulation slot after two ACT copies)
        kraw = sqpool.tile([P, Dm], BF16, name="kraw", tag="kraw")
        nc.scalar.activation(kraw, ps[:, 0:Dm], AF.Copy)
        nc.scalar.activation(vbig[:, ti * Dm:(ti + 1) * Dm], ps[:, Dm:2 * Dm],
                             AF.Copy)
        knb = qk_normalize(kraw, True)
        nc.gpsimd.dma_start(kn_dram[ti * P:(ti + 1) * P, :], knb)
        if ti % KQ == KQ - 1:
            h0 = (ti // KQ) * KQ * P
            hn = KQ * P
            for pr in range(PAIRS):
                nc.sync.dma_start_transpose(
                    knT[:, pr * Tkv + h0: pr * Tkv + h0 + hn],
                    kn_dram[h0:h0 + hn, pr * P:(pr + 1) * P])
        # interleave q token-blocks so the PE stream stays dense into phase B
        qtarget = (ti + 1) * TBq // TBkv
        while qdone < qtarget:
            emit_q(qdone)
            qdone += 1

    wxa.close()
    free_xkvT()
    free_wnT()

    # ---------------- phase B: scores -> sigmoid -> attn @ v ----------------
    # Software-pipelined: scores for unit i+1 are issued to the PE before the
    # attn@v of unit i, so the PE works under each sigmoid instead of stalling
    # in FIFO order behind it. unit = (pair, key-block, head-in-pair).
    avt_big, _ = _tile([P, PAIRS * Tq], BF16, "avt_big")
    bstk = ExitStack()
    psS = bstk.enter_context(tc.tile_pool(name="psS", bufs=3, space="PSUM"))
    psAV = bstk.enter_context(tc.tile_pool(name="psAV", bufs=1, space="PSUM"))
    attnp = bstk.enter_context(tc.tile_pool(name="attnp", bufs=6))

    # unit = (pair, key-block, t-half). One [128, 1024] score tile holds BOTH
    # heads' [128, 512] score blocks side by side: the two K=64 matmuls are
    # emitted adjacently (concurrent in disjoint PE row groups), and ONE
    # FD=1024 sigmoid covers both heads.
    THW = min(512, Tq)
    TH = Tq // THW
    units = [(pr, th, sb) for pr in range(PAIRS) for th in range(TH)
             for sb in range(TBkv)]
    psav_by_pair = {}
    pss_by_unit = {}

    def emit_scores(u):
        pr, th, sb = u
        pss = psS.tile([P, 2 * THW], F32, name="pss", tag="pss")
        pss_by_unit[u] = pss
        for a in (0, 1):
            r0 = a * HDl
            nc.tensor.matmul(
                pss[:, a * THW:(a + 1) * THW],
                lhsT=knT[r0:r0 + HDl, pr * Tkv + sb * P: pr * Tkv + (sb + 1) * P],
                rhs=qnT[r0:r0 + HDl, pr * Tq + th * THW: pr * Tq + (th + 1) * THW],
                start=True, stop=True)

    emit_scores(units[0])
    emit_scores(units[1])
    for i, u in enumerate(units):
        pr, th, sb = u
        if i + 2 < len(units):
            emit_scores(units[i + 2])
        if sb == 0 and th == 0:
            psav_by_pair[pr] = psAV.tile([P, Tq], F32, name="psav", tag="psav")
        psav = psav_by_pair[pr]
        pss = pss_by_unit.pop(u)
        attn = attnp.tile([P, 2 * THW], BF16, name="attn", tag="attn")
        nc.scalar.activation(attn, pss, AF.Sigmoid)
        for a in (0, 1):
            r0 = a * HDl
            nc.tensor.matmul(
                psav[r0:r0 + HDl, th * THW:(th + 1) * THW],
                lhsT=vbig[:, sb * Dm + pr * P + r0: sb * Dm + pr * P + r0 + HDl],
                rhs=attn[:, a * THW:(a + 1) * THW],
                start=(sb == 0), stop=(sb == TBkv - 1),
                skip_group_check=True)
        if sb == TBkv - 1 and th == TH - 1:
            nc.vector.tensor_copy(avt_big[:, pr * Tq:(pr + 1) * Tq], psav)
            # natural-layout transposes on the idle sync xbar ring (PE and
            # scalar are saturated during phase B)
            for tb in range(TBq):
                nc.sync.dma_start_transpose(
                    avnat[:, tb * Dm + pr * P: tb * Dm + (pr + 1) * P],
                    avt_big[:, pr * Tq + tb * P: pr * Tq + (tb + 1) * P])
    bstk.close()

    # ---------------- phase C: normalize + magnitude + out-proj ----------------
    avnT, _ = _tile([P, DT * Tq], BF16, "avnT")
    cstk = ExitStack()
    psO = cstk.enter_context(tc.tile_pool(name="psO", bufs=2, space="PSUM"))
    psT2 = cstk.enter_context(tc.tile_pool(name="psT2", bufs=4, space="PSUM"))
    sqc = cstk.enter_context(tc.tile_pool(name="sqc", bufs=4))
    smallc = cstk.enter_context(tc.tile_pool(name="smallc", bufs=24))
    avnp = cstk.enter_context(tc.tile_pool(name="avnp", bufs=4))
    ypool = cstk.enter_context(tc.tile_pool(name="ypool", bufs=3))

    def c_norm(tb):
        src = avnat[:, tb * Dm:(tb + 1) * Dm]
        sqa = sqc.tile([P, Dm], BF16, name="sqa", tag="sqa")
        nc.vector.tensor_mul(sqa, src, src)
        ssa = smallc.tile([P, Hn], F32, name="ssa", tag="sh")
        nc.vector.tensor_reduce(ssa, sqa.rearrange("p (h d) -> p h d", h=Hn),
                                axis=AX.X, op=ALU.add)
        sa = smallc.tile([P, Hn], F32, name="sa", tag="sh")
        nc.scalar.activation(sa, ssa, AF.Sqrt)
        sae = smallc.tile([P, Hn], F32, name="sae", tag="sh")
        nc.vector.tensor_scalar_add(sae, sa, eps_av)
        ra = smallc.tile([P, Hn], F32, name="ra", tag="sh")
        nc.vector.reciprocal(ra, sae)
        g = smallc.tile([P, Hn], F32, name="g", tag="sh")
        nc.vector.tensor_scalar_mul(g, ra, mag8[:, tb:tb + 1])
        avn = avnp.tile([P, Dm], BF16, name="avn", tag="avn")
        nc.vector.tensor_tensor(
            avn.rearrange("p (h d) -> p h d", h=Hn),
            src.rearrange("p (h d) -> p h d", h=Hn),
            g.broadcast_to([P, Hn, HDl]),
            op=ALU.mult)
        ptt = psT2.tile([P, DT * P], BF16, name="ptt2", tag="ptt2")
        for dt in range(DT):
            nc.tensor.transpose(ptt[:, dt * P:(dt + 1) * P],
                                avn[:, dt * P:(dt + 1) * P], ident)
        dst3 = avnT.rearrange("p (dt s) -> p dt s", dt=DT)[:, :, tb * P:(tb + 1) * P]
        nc.vector.tensor_copy(dst3, ptt.rearrange("p (dt s) -> p dt s", dt=DT))

    def c_proj(tb):
        pso = psO.tile([P, Dm], F32, name="pso", tag="pso")
        for dt in range(DT):
            lhs = avnT[:, dt * Tq + tb * P: dt * Tq + (tb + 1) * P]
            for (c0, cn) in _chunks(Dm, 512):
                nc.tensor.matmul(pso[:, c0:c0 + cn], lhsT=lhs,
                                 rhs=ownT[:, dt * Dm + c0: dt * Dm + c0 + cn],
                                 start=(dt == 0), stop=(dt == DT - 1))
        ysb = ypool.tile([P, Dm], F32, name="ysb", tag="ysb")
        nc.scalar.activation(ysb, pso, AF.Copy)
        nc.gpsimd.dma_start(y[tb * P:(tb + 1) * P, :], ysb)

    for tb in range(TBq + 1):
        if tb < TBq:
            c_norm(tb)
        if tb >= 1:
            c_proj(tb - 1)
    cstk.close()
    dstk.close()


def make_nc(Tq=T // 2, Tkv=T, Dm=D, Hn=H):
    nc = bacc.Bacc("TRN2", target_bir_lowering=False, debug=False,
                   num_devices=N_CORES)
    with ExitStack() as ctx:
        with tile.TileContext(nc) as tc:
            build_program(nc, tc, ctx, Tq, Tkv, Dm, Hn)
    nc.compile()
    return nc


_CACHED_NC = None


def _get_nc():
    global _CACHED_NC
    if _CACHED_NC is None:
        _CACHED_NC = make_nc()
    return _CACHED_NC


def _shard_inputs(x, qkv_w, out_w):
    Tq = T // 2
    x = np.asarray(x, dtype=np.float32)
    qkv_w = np.ascontiguousarray(np.asarray(qkv_w, dtype=np.float32))
    out_w = np.ascontiguousarray(np.asarray(out_w, dtype=np.float32))
    in_maps = []
    for core in range(N_CORES):
        b, half = core // 2, core % 2
        own = x[b, half * Tq:(half + 1) * Tq]
        other = x[b, (1 - half) * Tq:(2 - half) * Tq]
        xkv = np.ascontiguousarray(np.concatenate([own, other], axis=0))
        in_maps.append({"xkv": xkv, "qkvw": qkv_w, "outw": out_w})
    return in_maps


def run(x, qkv_w, out_w, trace=False, trace_cores=None):
    nc = _get_nc()
    in_maps = _shard_inputs(x, qkv_w, out_w)
    res = run_bass_kernel_spmd(nc, in_maps, list(range(N_CORES)),
                               trace=trace, trace_cores=trace_cores)
    Tq = T // 2
    y = np.empty((B, T, D), np.float32)
    for core, r in enumerate(res.results):
        b, half = core // 2, core % 2
        y[b, half * Tq:(half + 1) * Tq] = r["y"]
    return y, res


def kernel(x, qkv_w, out_w):
    y, _ = run(x, qkv_w, out_w, trace=False)
    return y


# revision 53
# speedup vs baseline: 1.1745x; 1.1745x over previous
"""Trainium2 Bass kernel for nn_Attention_4363686773373.

Sigmoid attention with magnitude-preserving (weight-normalized) projections.

Sharding: data-parallel over (batch, T-half) -> 8 shards on 8 NeuronCores.
Each core computes q for its 1024 tokens and k,v for the full 2048 tokens of
its batch (k/v recomputed on both cores; no collectives). Each core's xkv
rows are pre-ordered so its query tokens come first (attention is
permutation-invariant over the key axis), keeping the program SPMD-uniform.

Per-core dataflow (heavy matmuls in bf16 with fp32 PSUM accumulation):
  W/X: row-normalize qkv_w on device and PE-transpose it (and the bf16-cast
     x) into [d, .] layouts; out_w is normalized and bounced through DRAM
     with large DMA-xbar transposes (only needed by the out-projection);
     per-token ||x|| via ACT square+accumulate.
  A: qkv projection in natural [t, e] layout (lhsT = xT tiles), fast PSUM
     eviction through ACT copies, q/k cosine-normalization along head_dim
     via free-dim reduces, then DRAM-bounce transposes to [head_dim, t]
     layout. q token-blocks are interleaved into the k/v loop.
  B: unit = (head-pair, t-half, key-block), t-half-major so early units only
     need early qn transposes. Both heads' K=64 score matmuls land side by
     side in one [128, 1024] PSUM tile (adjacent issue -> concurrent in
     disjoint PE row groups); ONE FD=1024 sigmoid on the scalar engine
     yields bf16 attn weights; attn^T @ v accumulates per pair. Score tiles
     are triple-buffered and issued two units ahead so the PE never stalls
     inside a sigmoid (keeps the PE HAM clock warm -- the biggest lever
     observed: a cold-entering phase B stays at half clock for its entire
     duration). Per-pair attn-out transposes run on the idle sync xbar ring.
  C: software-pipelined per token-block: normalize per (token, head), scale
     by token magnitude, PE-transpose to [d, t], out-projection, store.

DMA rings: sync = loads + xbar transposes, scalar = weight loads, gpsimd =
DRAM scratch writes + output stores. ACT table sets: all sqrt-set work
strictly precedes the sigmoid phase; one switch back for phase C's sqrt
(2 table switches total). PE-transpose batches land in a single-bank PSUM
tile and are evicted by one strided DVE copy. Measured ~422 us on 8
axon-tunneled trn2 cores, rel err 4.5e-3 vs the fp32 reference.
"""

import math
from contextlib import ExitStack

import numpy as np

import concourse.bass as bass
import concourse.tile as tile
from concourse import bacc, mybir
from concourse.bass_utils import run_bass_kernel_spmd
from concourse.masks import make_identity

# Problem shapes (hardcoded per harness contract)
B, T, D, H = 4, 2048, 768, 12
HD = D // H  # 64
EPS = 1e-4
SIGMOID_GAIN = 1.8402
N_CORES = 8

F32 = mybir.dt.float32
BF16 = mybir.dt.bfloat16
AF = mybir.ActivationFunctionType
ALU = mybir.AluOpType
AX = mybir.AxisListType


def _ensure_axon_hooks():
    """This image's antenv lacks axon_hooks; reconstruct it so trace=True
    (NTFF profiling) works instead of crashing on import."""
    try:
        import antenv.axon_hooks  # noqa: F401
        return
    except ImportError:
        pass
    import sys
    import types
    try:
        import antenv
    except ImportError:
        return
    mod = types.ModuleType("antenv.axon_hooks")
    _hook = [None]
    mod.set_axon_ntff_profile_hook = lambda h: _hook.__setitem__(0, h)
    mod.get_axon_ntff_profile_hook = lambda: _hook[0]
    sys.modules["antenv.axon_hooks"] = mod
    antenv.axon_hooks = mod
    try:
        from trn_agent_boot.trn_boot import _ntff_profile_via_ctypes
        mod.set_axon_ntff_profile_hook(
            _ntff_profile_via_ctypes('/opt/axon/libaxon_pjrt.so'))
    except Exception:
        pass


_ensure_axon_hooks()

if __import__("os").environ.get("ANT_LDW_OPT") == "1":
    import concourse.bass_utils as _bu
    _orig_rc = _bu.run_command

    def _rc_ldw(argv, **kw):
        argv = ["--enable-ldw-opt=true" if a == "--enable-ldw-opt=false" else a
                for a in argv]
        return _orig_rc(argv, **kw)

    _bu.run_command = _rc_ldw


def _chunks(total, maxn=1024):
    out = []
    c0 = 0
    while c0 < total:
        cn = min(maxn, total - c0)
        out.append((c0, cn))
        c0 += cn
    return out


def build_program(nc, tc, ctx, Tq, Tkv, Dm, Hn):
    """Emit the per-core program. xkv rows are pre-ordered so the first Tq
    tokens are this core's query tokens (attention is permutation-invariant
    over the key axis)."""
    keep = []  # keep tc.tile free-closures alive (GC would release the pools)

    def _tile(shape, dtype, name):
        t, free = tc.tile(shape, dtype, name=name)
        keep.append(free)
        return t, free

    tc._ant_keepalive = keep
    P = 128
    HDl = 64
    assert Dm % P == 0 and Tq % P == 0 and Tkv % P == 0
    DT = Dm // P          # d-tiles
    E3 = 3 * Dm
    PAIRS = Hn // 2       # head pairs; pair = 128 contiguous features
    assert PAIRS * P == Dm and Hn * HDl == Dm
    TBq = Tq // P
    TBkv = Tkv // P
    WE = E3 // P          # qkv_w row tiles
    # eps seen by the post-attention normalize, after folding out the
    # gain/sqrt(T) prefactor (we accumulate raw attn@v).
    eps_av = EPS * math.sqrt(Tkv) / SIGMOID_GAIN

    xkv = nc.dram_tensor("xkv", [Tkv, Dm], F32, kind="ExternalInput").ap()
    qkvw = nc.dram_tensor("qkvw", [E3, Dm], F32, kind="ExternalInput").ap()
    outw = nc.dram_tensor("outw", [Dm, Dm], F32, kind="ExternalInput").ap()
    y = nc.dram_tensor("y", [Tq, Dm], F32, kind="ExternalOutput").ap()

    # ---------------- DRAM scratch ----------------
    dstk = ExitStack()
    dpool = dstk.enter_context(tc.tile_pool(name="dram", bufs=1, space="DRAM"))
    own_dram = dpool.tile([Dm, Dm], BF16, name="own_dram")
    kn_dram = dpool.tile([Tkv, Dm], BF16, name="kn_dram")
    qn_dram = dpool.tile([Tq, Dm], BF16, name="qn_dram")

    # ---------------- persistent SBUF tensors ----------------
    knT, _ = _tile([P, PAIRS * Tkv], BF16, "knT")    # [hd(2 heads), s]
    qnT, _ = _tile([P, PAIRS * Tq], BF16, "qnT")     # [hd(2 heads), t]
    vbig, _ = _tile([P, TBkv * Dm], BF16, "vbig")    # natural [s, e]
    mag8, _ = _tile([P, max(TBq, 2)], F32, "mag8")   # sqrt(||x||^2*HD/D)
    ownT, _ = _tile([P, DT * Dm], BF16, "ownT")      # out_w normalized^T
    avnat, _ = _tile([P, TBq * Dm], BF16, "avnat")   # attn-out natural
    ident, _ = _tile([P, P], BF16, "ident")          # PE-transpose identity
    make_identity(nc, ident)

    # ---------------- phase W + X + A (scoped) ----------------
    wxa = ExitStack()
    wnT, free_wnT = _tile([P, DT * E3], BF16, "wnT")
    xkvT, free_xkvT = _tile([P, DT * Tkv], BF16, "xkvT")
    wstage = wxa.enter_context(tc.tile_pool(name="wstage", bufs=6))
    sqpool = wxa.enter_context(tc.tile_pool(name="sqpool", bufs=4))
    small = wxa.enter_context(tc.tile_pool(name="small", bufs=24))
    nstage = wxa.enter_context(tc.tile_pool(name="nstage", bufs=6))
    psA = wxa.enter_context(tc.tile_pool(name="psA", bufs=2, space="PSUM"))
    psW = wxa.enter_context(tc.tile_pool(name="psW", bufs=2, space="PSUM"))

    def pe_transpose_cols(src, dst_big, cols, stride, base):
        """PE-transpose src [P, DT*P] column blocks into dst_big where block
        dt lands at dst_big[:, dt*stride + base : +cols]. All DT transposes
        land in one single-bank PSUM tile, evicted by ONE strided DVE copy."""
        ptw = psW.tile([P, DT * P], BF16, name="ptw", tag="ptw")
        for dt in range(DT):
            nc.tensor.transpose(ptw[:, dt * P:(dt + 1) * P],
                                src[:, dt * P:(dt + 1) * P], ident,
                                )
        dst3 = dst_big.rearrange("p (dt s) -> p dt s", dt=DT)[:, :, base:base + cols]
        nc.vector.tensor_copy(dst3, ptw.rearrange("p (dt s) -> p dt s", dt=DT))

    def normalize_w(we):
        """qkv_w row-tile we -> bf16 rows/(||row||+eps), PE-transposed into
        wnT."""
        wst = wstage.tile([P, Dm], F32, name="wst", tag="wst")
        nc.scalar.dma_start(wst, qkvw[we * P:(we + 1) * P, :])
        wsq = sqpool.tile([P, Dm], BF16, name="wsq", tag="sq")
        ssw = small.tile([P, 1], F32, name="ssw", tag="s1")
        nc.scalar.activation(wsq, wst, AF.Square, accum_out=ssw)
        sw = small.tile([P, 1], F32, name="sw", tag="s1")
        nc.scalar.activation(sw, ssw, AF.Sqrt)
        swe = small.tile([P, 1], F32, name="swe", tag="s1")
        nc.vector.tensor_scalar_add(swe, sw, EPS)
        rw = small.tile([P, 1], F32, name="rw", tag="s1")
        nc.vector.reciprocal(rw, swe)
        wnb = nstage.tile([P, Dm], BF16, name="wnb", tag="nst")
        nc.vector.tensor_scalar_mul(wnb, wst, rw)
        pe_transpose_cols(wnb, wnT, P, E3, we * P)

    def load_x(ti):
        """x token-block ti: magnitude, bf16 cast, PE-transpose into xkvT."""
        xst = wstage.tile([P, Dm], F32, name="xst", tag="wst")
        nc.sync.dma_start(xst, xkv[ti * P:(ti + 1) * P, :])
        if ti < TBq:
            xsq = sqpool.tile([P, Dm], BF16, name="xsq", tag="sq")
            ssx = small.tile([P, 1], F32, name="ssx", tag="s1")
            nc.scalar.activation(xsq, xst, AF.Square, accum_out=ssx)
            nc.scalar.activation(mag8[:, ti:ti + 1], ssx, AF.Sqrt,
                                 scale=float(HDl) / float(Dm))
        xbf = nstage.tile([P, Dm], BF16, name="xbf", tag="nst")
        nc.vector.tensor_copy(xbf, xst)
        pe_transpose_cols(xbf, xkvT, P, Tkv, ti * P)

    # interleave x blocks with k/v weight rows (rows Dm..3Dm); phase A's kv
    # loop needs all kv weight tiles + per-ti x tiles
    for i in range(max(TBkv, 2 * DT)):
        if i < TBkv:
            load_x(i)
        if i < 2 * DT:
            normalize_w(DT + i)
    for we in range(DT):     # q weight rows last (q loop runs after kv loop)
        normalize_w(we)

    # out-projection weights: normalize -> DRAM bounce -> xbar transpose
    # (only needed by phase C; uses idle DMA capacity during A/B)
    for we in range(DT):
        wst = wstage.tile([P, Dm], F32, name="wso", tag="wst")
        nc.scalar.dma_start(wst, outw[we * P:(we + 1) * P, :])
        wsq = sqpool.tile([P, Dm], BF16, name="wsqo", tag="sq")
        ssw = small.tile([P, 1], F32, name="sswo", tag="s1")
        nc.scalar.activation(wsq, wst, AF.Square, accum_out=ssw)
        sw = small.tile([P, 1], F32, name="swo", tag="s1")
        nc.scalar.activation(sw, ssw, AF.Sqrt)
        swe = small.tile([P, 1], F32, name="sweo", tag="s1")
        nc.vector.tensor_scalar_add(swe, sw, EPS)
        rw = small.tile([P, 1], F32, name="rwo", tag="s1")
        nc.vector.reciprocal(rw, swe)
        wnb = nstage.tile([P, Dm], BF16, name="wnbo", tag="nst")
        nc.vector.tensor_scalar_mul(wnb, wst, rw)
        nc.gpsimd.dma_start(own_dram[we * P:(we + 1) * P, :], wnb)
    for dt in range(DT):
        nc.sync.dma_start_transpose(
            ownT[:, dt * Dm:(dt + 1) * Dm],
            own_dram[:, dt * P:(dt + 1) * P])

    # qkv projection + q/k normalization, natural layout
    def qk_normalize(kraw, is_k):
        """kraw: SBUF bf16 [P, Dm] raw q or k; returns normalized bf16 tile."""
        sqk = sqpool.tile([P, Dm], BF16, name="sqk", tag="sq")
        nc.vector.tensor_mul(sqk, kraw, kraw)
        ssk = small.tile([P, Hn], F32, name="ssk", tag="sh")
        nc.vector.tensor_reduce(ssk, sqk.rearrange("p (h d) -> p h d", h=Hn),
                                axis=AX.X, op=ALU.add)
        sk = small.tile([P, Hn], F32, name="sk", tag="sh")
        nc.scalar.activation(sk, ssk, AF.Sqrt)
        ske = small.tile([P, Hn], F32, name="ske", tag="sh")
        if is_k:
            # fold the 1/sqrt(HD) score scale into k: sqrt(HD)/(||k||+eps)
            nc.vector.tensor_scalar(ske, sk, EPS, 1.0 / math.sqrt(HDl),
                                    op0=ALU.add, op1=ALU.mult)
        else:
            nc.vector.tensor_scalar_add(ske, sk, EPS)
        rk = small.tile([P, Hn], F32, name="rk", tag="sh")
        nc.vector.reciprocal(rk, ske)
        knb = nstage.tile([P, Dm], BF16, name="knb", tag="nst")
        nc.vector.tensor_tensor(
            knb.rearrange("p (h d) -> p h d", h=Hn),
            kraw.rearrange("p (h d) -> p h d", h=Hn),
            rk.broadcast_to([P, Hn, HDl]),
            op=ALU.mult)
        return knb

    def emit_q(ti):
        # q for this core's token blocks (first TBq blocks of xkv)
        ps = psA.tile([P, Dm], F32, name="psq", tag="ps")
        for dt in range(DT):
            lhs = xkvT[:, dt * Tkv + ti * P: dt * Tkv + (ti + 1) * P]
            for (c0, cn) in _chunks(Dm, 512):
                nc.tensor.matmul(ps[:, c0:c0 + cn], lhsT=lhs,
                                 rhs=wnT[:, dt * E3 + c0: dt * E3 + c0 + cn],
                                 start=(dt == 0), stop=(dt == DT - 1))
        qraw = sqpool.tile([P, Dm], BF16, name="qraw", tag="kraw")
        nc.scalar.activation(qraw, ps[:, 0:Dm], AF.Copy)
        qnb = qk_normalize(qraw, False)
        nc.gpsimd.dma_start(qn_dram[ti * P:(ti + 1) * P, :], qnb)
        QH = max(TBq // 2, 1)
        if ti % QH == QH - 1:
            h0 = (ti // QH) * QH * P
            hn = QH * P
            for pr in range(PAIRS):
                nc.sync.dma_start_transpose(
                    qnT[:, pr * Tq + h0: pr * Tq + h0 + hn],
                    qn_dram[h0:h0 + hn, pr * P:(pr + 1) * P])

    KQ = max(TBkv // 4, 1)
    qdone = 0
    for ti in range(TBkv):
        # k,v for every token block
        ps = psA.tile([P, 2 * Dm], F32, name="pskv", tag="ps")
        for dt in range(DT):
            lhs = xkvT[:, dt * Tkv + ti * P: dt * Tkv + (ti + 1) * P]
            for (c0, cn) in _chunks(2 * Dm, 512):
                nc.tensor.matmul(ps[:, c0:c0 + cn], lhsT=lhs,
                                 rhs=wnT[:, dt * E3 + Dm + c0: dt * E3 + Dm + c0 + cn],
                                 start=(dt == 0), stop=(dt == DT - 1))
        # evict PSUM quickly (frees the accumulation slot after two ACT copies)
        kraw = sqpool.tile([P, Dm], BF16, name="kraw", tag="kraw")
        nc.scalar.activation(kraw, ps[:, 0:Dm], AF.Copy)
        nc.scalar.activation(vbig[:, ti * Dm:(ti + 1) * Dm], ps[:, Dm:2 * Dm],
                             AF.Copy)
        knb = qk_normalize(kraw, True)
        nc.gpsimd.dma_start(kn_dram[ti * P:(ti + 1) * P, :], knb)
        if ti % KQ == KQ - 1:
            h0 = (ti // KQ) * KQ * P
            hn = KQ * P
            for pr in range(PAIRS):
                nc.sync.dma_start_transpose(
                    knT[:, pr * Tkv + h0: pr * Tkv + h0 + hn],
                    kn_dram[h0:h0 + hn, pr * P:(pr + 1) * P])
        # interleave q token-blocks so the PE stream stays dense into phase B
        qtarget = (ti + 1) * TBq // TBkv
        while qdone < qtarget:
            emit_q(qdone)
            qdone += 1

    wxa.close()
    free_xkvT()
    free_wnT()

    # ---------------- phase B: scores -> sigmoid -> attn @ v ----------------
    # Software-pipelined: scores for unit i+1 are issued to the PE before the
    # attn@v of unit i, so the PE works under each sigmoid instead of stalling
    # in FIFO order behind it. unit = (pair, key-block, head-in-pair).
    avt_big, _ = _tile([P, PAIRS * Tq], BF16, "avt_big")
    bstk = ExitStack()
    psS = bstk.enter_context(tc.tile_pool(name="psS", bufs=3, space="PSUM"))
    psAV = bstk.enter_context(tc.tile_pool(name="psAV", bufs=1, space="PSUM"))
    attnp = bstk.enter_context(tc.tile_pool(name="attnp", bufs=6))

    # unit = (pair, key-block, t-half). One [128, 1024] score tile holds BOTH
    # heads' [128, 512] score blocks side by side: the two K=64 matmuls are
    # emitted adjacently (concurrent in disjoint PE row groups), and ONE
    # FD=1024 sigmoid covers both heads.
    THW = min(512, Tq)
    TH = Tq // THW
    units = [(pr, th, sb) for pr in range(PAIRS) for th in range(TH)
             for sb in range(TBkv)]
    psav_by_pair = {}
    pss_by_unit = {}

    def emit_scores(u):
        pr, th, sb = u
        pss = psS.tile([P, 2 * THW], F32, name="pss", tag="pss")
        pss_by_unit[u] = pss
        for a in (0, 1):
            r0 = a * HDl
            nc.tensor.matmul(
                pss[:, a * THW:(a + 1) * THW],
                lhsT=knT[r0:r0 + HDl, pr * Tkv + sb * P: pr * Tkv + (sb + 1) * P],
                rhs=qnT[r0:r0 + HDl, pr * Tq + th * THW: pr * Tq + (th + 1) * THW],
                start=True, stop=True)

    emit_scores(units[0])
    emit_scores(units[1])
    for i, u in enumerate(units):
        pr, th, sb = u
        if i + 2 < len(units):
            emit_scores(units[i + 2])
        if sb == 0 and th == 0:
            psav_by_pair[pr] = psAV.tile([P, Tq], F32, name="psav", tag="psav")
        psav = psav_by_pair[pr]
        pss = pss_by_unit.pop(u)
        attn = attnp.tile([P, 2 * THW], BF16, name="attn", tag="attn")
        nc.scalar.activation(attn, pss, AF.Sigmoid)
        for a in (0, 1):
            r0 = a * HDl
            nc.tensor.matmul(
                psav[r0:r0 + HDl, th * THW:(th + 1) * THW],
                lhsT=vbig[:, sb * Dm + pr * P + r0: sb * Dm + pr * P + r0 + HDl],
                rhs=attn[:, a * THW:(a + 1) * THW],
                start=(sb == 0), stop=(sb == TBkv - 1),
                skip_group_check=True)
        if sb == TBkv - 1:
            # this t-half's attn-out is complete (t-half-major order): evict
            # it and run its natural-layout transposes on the idle sync xbar
            # ring now, shrinking the B->C boundary bubble
            c0 = th * THW
            nc.vector.tensor_copy(
                avt_big[:, pr * Tq + c0: pr * Tq + c0 + THW],
                psav[:, c0:c0 + THW])
            for tb in range(c0 // P, (c0 + THW) // P):
                nc.sync.dma_start_transpose(
                    avnat[:, tb * Dm + pr * P: tb * Dm + (pr + 1) * P],
                    avt_big[:, pr * Tq + tb * P: pr * Tq + (tb + 1) * P])
    bstk.close()

    # ---------------- phase C: normalize + magnitude + out-proj ----------------
    avnT, _ = _tile([P, DT * Tq], BF16, "avnT")
    cstk = ExitStack()
    psO = cstk.enter_context(tc.tile_pool(name="psO", bufs=2, space="PSUM"))
    psT2 = cstk.enter_context(tc.tile_pool(name="psT2", bufs=4, space="PSUM"))
    sqc = cstk.enter_context(tc.tile_pool(name="sqc", bufs=4))
    smallc = cstk.enter_context(tc.tile_pool(name="smallc", bufs=24))
    avnp = cstk.enter_context(tc.tile_pool(name="avnp", bufs=4))
    ypool = cstk.enter_context(tc.tile_pool(name="ypool", bufs=3))

    def c_norm(tb):
        src = avnat[:, tb * Dm:(tb + 1) * Dm]
        sqa = sqc.tile([P, Dm], BF16, name="sqa", tag="sqa")
        nc.vector.tensor_mul(sqa, src, src)
        ssa = smallc.tile([P, Hn], F32, name="ssa", tag="sh")
        nc.vector.tensor_reduce(ssa, sqa.rearrange("p (h d) -> p h d", h=Hn),
                                axis=AX.X, op=ALU.add)
        sa = smallc.tile([P, Hn], F32, name="sa", tag="sh")
        nc.scalar.activation(sa, ssa, AF.Sqrt)
        sae = smallc.tile([P, Hn], F32, name="sae", tag="sh")
        nc.vector.tensor_scalar_add(sae, sa, eps_av)
        ra = smallc.tile([P, Hn], F32, name="ra", tag="sh")
        nc.vector.reciprocal(ra, sae)
        g = smallc.tile([P, Hn], F32, name="g", tag="sh")
        nc.vector.tensor_scalar_mul(g, ra, mag8[:, tb:tb + 1])
        avn = avnp.tile([P, Dm], BF16, name="avn", tag="avn")
        nc.vector.tensor_tensor(
            avn.rearrange("p (h d) -> p h d", h=Hn),
            src.rearrange("p (h d) -> p h d", h=Hn),
            g.broadcast_to([P, Hn, HDl]),
            op=ALU.mult)
        ptt = psT2.tile([P, DT * P], BF16, name="ptt2", tag="ptt2")
        for dt in range(DT):
            nc.tensor.transpose(ptt[:, dt * P:(dt + 1) * P],
                                avn[:, dt * P:(dt + 1) * P], ident)
        dst3 = avnT.rearrange("p (dt s) -> p dt s", dt=DT)[:, :, tb * P:(tb + 1) * P]
        nc.vector.tensor_copy(dst3, ptt.rearrange("p (dt s) -> p dt s", dt=DT))

    def c_proj(tb):
        pso = psO.tile([P, Dm], F32, name="pso", tag="pso")
        for dt in range(DT):
            lhs = avnT[:, dt * Tq + tb * P: dt * Tq + (tb + 1) * P]
            for (c0, cn) in _chunks(Dm, 512):
                nc.tensor.matmul(pso[:, c0:c0 + cn], lhsT=lhs,
                                 rhs=ownT[:, dt * Dm + c0: dt * Dm + c0 + cn],
                                 start=(dt == 0), stop=(dt == DT - 1))
        ysb = ypool.tile([P, Dm], F32, name="ysb", tag="ysb")
        nc.scalar.activation(ysb, pso, AF.Copy)
        nc.gpsimd.dma_start(y[tb * P:(tb + 1) * P, :], ysb)

    for tb in range(TBq + 1):
        if tb < TBq:
            c_norm(tb)
        if tb >= 1:
            c_proj(tb - 1)
    cstk.close()
    dstk.close()


def make_nc(Tq=T // 2, Tkv=T, Dm=D, Hn=H):
    nc = bacc.Bacc("TRN2", target_bir_lowering=False, debug=False,
                   num_devices=N_CORES)
    with ExitStack() as ctx:
        with tile.TileContext(nc) as tc:
            build_program(nc, tc, ctx, Tq, Tkv, Dm, Hn)
    nc.compile()
    return nc


_CACHED_NC = None


def _get_nc():
    global _CACHED_NC
    if _CACHED_NC is None:
        _CACHED_NC = make_nc()
    return _CACHED_NC


def _shard_inputs(x, qkv_w, out_w):
    Tq = T // 2
    x = np.asarray(x, dtype=np.float32)
    qkv_w = np.ascontiguousarray(np.asarray(qkv_w, dtype=np.float32))
    out_w = np.ascontiguousarray(np.asarray(out_w, dtype=np.float32))
    in_maps = []
    for core in range(N_CORES):
        b, half = core // 2, core % 2
        own = x[b, half * Tq:(half + 1) * Tq]
        other = x[b, (1 - half) * Tq:(2 - half) * Tq]
        xkv = np.ascontiguousarray(np.concatenate([own, other], axis=0))
        in_maps.append({"xkv": xkv, "qkvw": qkv_w, "outw": out_w})
    return in_maps


def run(x, qkv_w, out_w, trace=False, trace_cores=None):
    nc = _get_nc()
    in_maps = _shard_inputs(x, qkv_w, out_w)
    res = run_bass_kernel_spmd(nc, in_maps, list(range(N_CORES)),
                               trace=trace, trace_cores=trace_cores)
    Tq = T // 2
    y = np.empty((B, T, D), np.float32)
    for core, r in enumerate(res.results):
        b, half = core // 2, core % 2
        y[b, half * Tq:(half + 1) * Tq] = r["y"]
    return y, res


def kernel(x, qkv_w, out_w):
    y, _ = run(x, qkv_w, out_w, trace=False)
    return y
